# revision 17
# baseline (speedup 1.0000x reference)
"""AFNO layer (2D rFFT -> block-diag complex MLP -> softshrink -> irFFT -> +skip)
as a Bass/Tile kernel on 8 TRN2 NeuronCores.

Sharding: the num_blocks axis (NB=8 blocks of 96 channels) maps one block per
core -- the FFTs are per-channel over spatial dims and the MLP mixes only
within a block, so the 8 cores are fully independent (no collectives).

All DFTs are dense matmuls against precomputed (host-side) DFT matrices in
bf16; accumulation is fp32 in PSUM.  Every stage is laid out so the tensor
engine contraction dim (SBUF partition dim) chains through the pipeline:

  x[w,(h,d)] --S1(rfft_W)--> [h,(d,f)] --S2(fft_H)--> [d,(f,g)]
    --MLP1--> [o,(f,g)] --MLP2(+b2)--> [g,(f,{vr|vi})] --softshrink-->
    --invH--> [f,(h,d)] --invW(+skip)--> out[w,(h,d)]
"""

import numpy as np
import ml_dtypes

B = 4
H = 128
W = 128
D = 768
BS = 96          # block size = channels per core
F = 65           # rfft bins along W
NCORES = 8
TH = 0.01        # softshrink threshold
FG = F * 128     # positions per (f,g) plane

_CACHE = {}


def _make_consts(w1r, w1i, b1, w2r, w2i, b2):
    """Host-side constant matrices, keyed as the kernel's dram inputs."""
    bf = ml_dtypes.bfloat16
    th = 2 * np.pi / 128
    j = np.arange(128)
    f = np.arange(F)
    Cw = np.cos(th * np.outer(f, j)) / np.sqrt(128.0)
    Sw = np.sin(th * np.outer(f, j)) / np.sqrt(128.0)
    rw = np.concatenate([Cw.T, -Sw.T], axis=1)            # [128(w),130]
    Ch = np.cos(th * np.outer(j, j)) / np.sqrt(128.0)
    Sh = np.sin(th * np.outer(j, j)) / np.sqrt(128.0)
    rh1 = np.concatenate([Ch, -Sh], axis=1)               # [128(h),256] pairs XR
    rh2 = np.concatenate([Sh, Ch], axis=1)                # pairs XI
    rm1 = np.concatenate(
        [np.concatenate([w2r.T, w2i.T], axis=1),
         np.concatenate([b2[:, 0], b2[:, 1]])[None, :]], axis=0)   # [97,192]
    rm2 = np.concatenate(
        [np.concatenate([-w2i.T, w2r.T], axis=1),
         np.zeros((1, 192), np.float32)], axis=0)
    g1 = np.concatenate([Ch, Sh], axis=1)                 # [128(g),256] pairs YR
    g2 = np.concatenate([-Sh, Ch], axis=1)                # pairs YI
    cf = np.full(F, 2.0)
    cf[0] = 1.0
    cf[64] = 1.0
    art = (cf[None, :] * np.cos(th * np.outer(j, f)) / np.sqrt(128.0)).T  # [65,128]
    ait = (-cf[None, :] * np.sin(th * np.outer(j, f)) / np.sqrt(128.0)).T
    c16 = lambda a: np.ascontiguousarray(a).astype(bf)
    return {
        "rw": c16(rw), "rh1": c16(rh1), "rh2": c16(rh2),
        "w1rt": c16(w1r.T), "w1it": c16(w1i.T), "nw1it": c16(-w1i.T),
        "rm1": c16(rm1), "rm2": c16(rm2),
        "g1": c16(g1), "g2": c16(g2), "art": c16(art), "ait": c16(ait),
        "b1r": np.ascontiguousarray(b1[:, 0:1]).astype(np.float32),
        "b1i": np.ascontiguousarray(b1[:, 1:2]).astype(np.float32),
    }


def _build_kernel(ctx, tc, dram):
    import concourse.mybir as mybir

    nc = tc.nc
    bf = mybir.dt.bfloat16
    f32 = mybir.dt.float32
    AF = mybir.ActivationFunctionType
    OP = mybir.AluOpType

    xr = dram["xbf"].ap().rearrange("b h w d -> b w h d")     # [4,128(w),128(h),96]
    outr = dram["out"].ap().rearrange("b h w d -> b w h d")

    consts = ctx.enter_context(tc.tile_pool(name="consts", bufs=1))
    xin = ctx.enter_context(tc.tile_pool(name="xin", bufs=2))
    stg = ctx.enter_context(tc.tile_pool(name="stg", bufs=1))
    hpool = ctx.enter_context(tc.tile_pool(name="hpool", bufs=1))
    apool = ctx.enter_context(tc.tile_pool(name="apool", bufs=2))
    opool = ctx.enter_context(tc.tile_pool(name="opool", bufs=1))
    pp = ctx.enter_context(tc.tile_pool(name="ps", bufs=2, space="PSUM"))

    def cload(name, shape, dtype=bf):
        t = consts.tile(shape, dtype, tag=name)
        nc.sync.dma_start(out=t[:], in_=dram[name].ap())
        return t

    RW = cload("rw", [128, 130])
    RH1 = cload("rh1", [128, 256])
    RH2 = cload("rh2", [128, 256])
    W1RT = cload("w1rt", [96, 96])
    W1IT = cload("w1it", [96, 96])
    NW1IT = cload("nw1it", [96, 96])
    RM1 = cload("rm1", [97, 192])
    RM2 = cload("rm2", [97, 192])
    G1 = cload("g1", [128, 256])
    G2 = cload("g2", [128, 256])
    ART = cload("art", [65, 128])
    AIT = cload("ait", [65, 128])
    B1R = cload("b1r", [96, 1], f32)
    B1I = cload("b1i", [96, 1], f32)

    # persistent MLP hidden tiles with the bias ones-row (row 96)
    HRe = hpool.tile([97, F, 128], bf, tag="hre")
    HIe = hpool.tile([97, F, 128], bf, tag="hie")
    nc.vector.memset(HRe[96:97, :, :], 1.0)
    nc.vector.memset(HIe[96:97, :, :], 0.0)
    NTH = consts.tile([128, 1], f32, tag="nth")   # softshrink -t bias column
    nc.vector.memset(NTH[:, :], -TH)

    # weighted ACT/DVE load balancing for PSUM->SBUF evictions
    eng_ns = {"act": 0.0, "dve": 0.0}

    def evict(dst, src, fd):
        act_cost = (fd + 344) / 1.2
        dve_cost = (fd + 240) / 0.96
        if eng_ns["act"] + act_cost <= eng_ns["dve"] + dve_cost:
            eng_ns["act"] += act_cost
            nc.scalar.activation(out=dst, in_=src, func=AF.Copy)
        else:
            eng_ns["dve"] += dve_cost
            nc.vector.tensor_copy(out=dst, in_=src)

    for b in range(B):
        X0 = xin.tile([128, H, BS], bf, tag="x0")       # [w,(h,d)]
        for hc in range(4):
            nc.sync.dma_start(out=X0[:, hc * 32:(hc + 1) * 32, :],
                              in_=xr[b, :, hc * 32:(hc + 1) * 32, :])
        X0f = X0[:, :, :].rearrange("p h d -> p (h d)")

        # ---- S1: rfft along W.  per d: psum[h,130] = X0[:,:,d].T @ RW
        S1o = stg.tile([128, BS, 130], bf, tag="A")      # [h,(d,{fr|fi})]
        for grp in range(8):                             # 12 d per psum tile
            ps = pp.tile([128, 4, 512], f32, tag="ps")
            for jb in range(4):
                for k in range(3):
                    d = grp * 12 + jb * 3 + k
                    nc.tensor.matmul(ps[:, jb, k * 130:(k + 1) * 130],
                                     X0[:, :, d], RW[:, :],
                                     start=True, stop=True)
            evict(S1o[:, grp * 12:(grp + 1) * 12, :].rearrange(
                      "p (jb k) c -> p jb (k c)", jb=4),
                  ps[:, :, 0:390], 1560)

        # ---- S2: full fft along H. per f: psum[d,256] = XR_f.T@RH1 + XI_f.T@RH2
        ZR = stg.tile([96, F, 128], bf, tag="C")         # [d,(f,g)]
        ZI = stg.tile([96, F, 128], bf, tag="B")
        for grp in range(9):                             # 8 f per psum tile
            nf = min(8, F - grp * 8)
            ps = pp.tile([128, 4, 512], f32, tag="ps")
            for k in range(nf):
                f = grp * 8 + k
                sl = ps[0:96, k // 2, (k % 2) * 256:(k % 2) * 256 + 256]
                nc.tensor.matmul(sl, S1o[:, :, f], RH1[:, :],
                                 start=True, stop=False)
                nc.tensor.matmul(sl, S1o[:, :, 65 + f], RH2[:, :],
                                 start=False, stop=True)
            nbank = (nf + 1) // 2
            kin = min(2, nf)
            src = ps[0:96, :, :].rearrange("p jb (k g) -> p jb k g", k=2)
            dstR = ZR[:, grp * 8:grp * 8 + nf, :].rearrange(
                "p (jb k) g -> p jb k g", k=kin)
            dstI = ZI[:, grp * 8:grp * 8 + nf, :].rearrange(
                "p (jb k) g -> p jb k g", k=kin)
            evict(dstR, src[:, 0:nbank, 0:kin, 0:128], nf * 128)
            evict(dstI, src[:, 0:nbank, 0:kin, 128:256], nf * 128)

        # ---- MLP1 (+bias +relu): contract d. psum[o,cw] over col chunks
        ZRf = ZR[:, :, :].rearrange("p f g -> p (f g)")
        ZIf = ZI[:, :, :].rearrange("p f g -> p (f g)")
        HRf = HRe[0:96, :, :].rearrange("p f g -> p (f g)")
        HIf = HIe[0:96, :, :].rearrange("p f g -> p (f g)")
        nchunk = (FG + 511) // 512                        # 17
        for cp in range(0, nchunk, 2):
            ps = pp.tile([128, 4, 512], f32, tag="ps")
            for ci, c in enumerate(range(cp, min(cp + 2, nchunk))):
                c0 = c * 512
                cw = min(512, FG - c0)
                pr = ps[0:96, 2 * ci, 0:cw]
                pi = ps[0:96, 2 * ci + 1, 0:cw]
                nc.tensor.matmul(pr, W1RT[:, :], ZRf[:, c0:c0 + cw],
                                 start=True, stop=False)
                nc.tensor.matmul(pr, NW1IT[:, :], ZIf[:, c0:c0 + cw],
                                 start=False, stop=True)
                nc.tensor.matmul(pi, W1IT[:, :], ZRf[:, c0:c0 + cw],
                                 start=True, stop=False)
                nc.tensor.matmul(pi, W1RT[:, :], ZIf[:, c0:c0 + cw],
                                 start=False, stop=True)
                nc.scalar.activation(out=HRf[:, c0:c0 + cw], in_=pr,
                                     func=AF.Relu, bias=B1R[:, :], scale=1.0)
                eng_ns["act"] += (cw + 344) / 1.2
                nc.vector.tensor_scalar(out=HIf[:, c0:c0 + cw], in0=pi,
                                        scalar1=B1I[:, :], scalar2=0.0,
                                        op0=OP.add, op1=OP.max)
                eng_ns["dve"] += (cw + 240) / 0.96

        # ---- MLP2 (+b2 via ones-row) fused with softshrink:
        #      v = HRe_f.T@RM1 + HIe_f.T@RM2  (PSUM)
        #      a = relu(v - t)   [ACT];   y = min(v + t, a)   [DVE]
        Y = stg.tile([128, F, 192], bf, tag="C")          # [g,(f,{yr|yi})]
        for grp in range(9):
            nf = min(8, F - grp * 8)
            ps = pp.tile([128, 4, 512], f32, tag="ps")
            for k in range(nf):
                f = grp * 8 + k
                sl = ps[:, k // 2, (k % 2) * 256:(k % 2) * 256 + 192]
                nc.tensor.matmul(sl, HRe[:, f, :], RM1[:, :],
                                 start=True, stop=False)
                nc.tensor.matmul(sl, HIe[:, f, :], RM2[:, :],
                                 start=False, stop=True)
            nbank = (nf + 1) // 2
            kin = min(2, nf)
            src = ps[:, 0:nbank, :].rearrange(
                "p jb (k c) -> p jb k c", k=2)[:, :, 0:kin, 0:192]
            As = apool.tile([128, 8, 192], bf, tag="ash")
            adst = As[:, 0:nf, :].rearrange("p (jb k) c -> p jb k c", k=kin)
            nc.scalar.activation(out=adst, in_=src, func=AF.Relu,
                                 bias=NTH[:, :], scale=1.0)
            eng_ns["act"] += (nf * 192 + 344) / 1.2
            nc.vector.scalar_tensor_tensor(
                out=Y[:, grp * 8:grp * 8 + nf, :].rearrange(
                    "p (jb k) c -> p jb k c", k=kin),
                in0=src, scalar=TH, in1=adst, op0=OP.add, op1=OP.min)
            eng_ns["dve"] += (nf * 192 + 240) / 0.96

        # ---- invH: per d: psum[f,256] = YR_d.T@G1 + YI_d.T@G2
        YspR = stg.tile([65, H, BS], bf, tag="B")         # [f,(h,d)]
        YspI = stg.tile([65, H, BS], bf, tag="A")
        for grp in range(12):                             # 8 d per psum tile
            ps = pp.tile([128, 4, 512], f32, tag="ps")
            for k in range(8):
                d = grp * 8 + k
                sl = ps[0:65, k // 2, (k % 2) * 256:(k % 2) * 256 + 256]
                nc.tensor.matmul(sl, Y[:, :, d], G1[:, :],
                                 start=True, stop=False)
                nc.tensor.matmul(sl, Y[:, :, 96 + d], G2[:, :],
                                 start=False, stop=True)
            src = ps[0:65, :, :].rearrange("p jb (k c) -> p jb k c", k=2)
            d0 = grp * 8
            dstR = YspR[:, :, d0:d0 + 8].rearrange("p h (j k) -> p j k h", j=4)
            dstI = YspI[:, :, d0:d0 + 8].rearrange("p h (j k) -> p j k h", j=4)
            evict(dstR, src[:, :, :, 0:128], 1024)
            evict(dstI, src[:, :, :, 128:256], 1024)

        # ---- invW (+skip): psum[w,cw] = ART.T@YspR + AIT.T@YspI ; out += x
        YRf = YspR[:, :, :].rearrange("p h d -> p (h d)")
        YIf = YspI[:, :, :].rearrange("p h d -> p (h d)")
        outt = opool.tile([128, H, BS], bf, tag="outt")   # [w,(h,d)]
        outf = outt[:, :, :].rearrange("p h d -> p (h d)")
        for cp in range(6):                               # 4 chunks of 512 per tile
            ps = pp.tile([128, 4, 512], f32, tag="ps")
            for j in range(4):
                c0 = (cp * 4 + j) * 512
                nc.tensor.matmul(ps[:, j, :], ART[:, :], YRf[:, c0:c0 + 512],
                                 start=True, stop=False)
                nc.tensor.matmul(ps[:, j, :], AIT[:, :], YIf[:, c0:c0 + 512],
                                 start=False, stop=True)
            for j in range(0, 4, 2):
                c0 = (cp * 4 + j) * 512
                nc.vector.tensor_tensor(
                    out=outf[:, c0:c0 + 1024],
                    in0=ps[:, j:j + 2, :].rearrange("p j c -> p (j c)"),
                    in1=X0f[:, c0:c0 + 1024], op=OP.add)
                eng_ns["dve"] += (1024 + 240) / 0.96
        for hc in range(4):
            nc.sync.dma_start(out=outr[b, :, hc * 32:(hc + 1) * 32, :],
                              in_=outt[:, hc * 32:(hc + 1) * 32, :])


def _get_compiled():
    if "nc" in _CACHE:
        return _CACHE["nc"]
    import concourse.mybir as mybir
    import concourse.tile as tile
    from concourse import bacc

    nc = bacc.Bacc("TRN2", target_bir_lowering=False, debug=False)
    bf = mybir.dt.bfloat16
    f32 = mybir.dt.float32
    dram = {}
    dram["xbf"] = nc.dram_tensor("xbf", [B, H, W, BS], bf, kind="ExternalInput")
    for name, shape in [("rw", [128, 130]), ("rh1", [128, 256]),
                        ("rh2", [128, 256]), ("w1rt", [96, 96]),
                        ("w1it", [96, 96]), ("nw1it", [96, 96]),
                        ("rm1", [97, 192]), ("rm2", [97, 192]),
                        ("g1", [128, 256]), ("g2", [128, 256]),
                        ("art", [65, 128]), ("ait", [65, 128])]:
        dram[name] = nc.dram_tensor(name, shape, bf, kind="ExternalInput")
    dram["b1r"] = nc.dram_tensor("b1r", [96, 1], f32, kind="ExternalInput")
    dram["b1i"] = nc.dram_tensor("b1i", [96, 1], f32, kind="ExternalInput")
    dram["out"] = nc.dram_tensor("out", [B, H, W, BS], bf, kind="ExternalOutput")

    from contextlib import ExitStack
    with tile.TileContext(nc) as tc:
        with ExitStack() as ctx:
            _build_kernel(ctx, tc, dram)
    nc.compile()
    _CACHE["nc"] = nc
    return nc


LAST_RESULT = None


def kernel(x, w1r, w1i, b1, w2r, w2i, b2):
    global LAST_RESULT
    from concourse.bass_utils import run_bass_kernel_spmd

    x = np.asarray(x)
    consts = _make_consts(np.asarray(w1r, np.float32), np.asarray(w1i, np.float32),
                          np.asarray(b1, np.float32), np.asarray(w2r, np.float32),
                          np.asarray(w2i, np.float32), np.asarray(b2, np.float32))
    nc = _get_compiled()
    in_maps = []
    for c in range(NCORES):
        m = dict(consts)
        m["xbf"] = np.ascontiguousarray(
            x[:, :, :, c * BS:(c + 1) * BS]).astype(ml_dtypes.bfloat16)
        in_maps.append(m)
    res = run_bass_kernel_spmd(nc, in_maps, core_ids=list(range(NCORES)))
    LAST_RESULT = res
    out = np.concatenate(
        [res.results[c]["out"].astype(np.float32) for c in range(NCORES)], axis=3)
    return out


# revision 18
# speedup vs baseline: 1.3170x; 1.3170x over previous
"""AFNO layer (2D rFFT -> block-diag complex MLP -> softshrink -> irFFT -> +skip)
as a Bass/Tile kernel on 8 TRN2 NeuronCores.

Sharding: the num_blocks axis (NB=8 blocks of 96 channels) maps one block per
core -- the FFTs are per-channel over spatial dims and the MLP mixes only
within a block, so the 8 cores are fully independent (no collectives).

All DFTs are dense matmuls against precomputed (host-side) DFT matrices in
bf16; accumulation is fp32 in PSUM.  Every stage is laid out so the tensor
engine contraction dim (SBUF partition dim) chains through the pipeline:

  x[w,(h,d)] --S1(rfft_W)--> [h,(d,f)] --S2(fft_H)--> [d,(f,g)]
    --MLP1--> [o,(f,g)] --MLP2(+b2)--> [g,(f,{vr|vi})] --softshrink-->
    --invH--> [f,(h,d)] --invW(+skip)--> out[w,(h,d)]
"""

import numpy as np
import ml_dtypes

B = 4
H = 128
W = 128
D = 768
BS = 96          # block size = channels per core
F = 65           # rfft bins along W
NCORES = 8
TH = 0.01        # softshrink threshold
FG = F * 128     # positions per (f,g) plane

_CACHE = {}


def _make_consts(w1r, w1i, b1, w2r, w2i, b2):
    """Host-side constant matrices, keyed as the kernel's dram inputs."""
    bf = ml_dtypes.bfloat16
    th = 2 * np.pi / 128
    j = np.arange(128)
    f = np.arange(F)
    Cw = np.cos(th * np.outer(f, j)) / np.sqrt(128.0)
    Sw = np.sin(th * np.outer(f, j)) / np.sqrt(128.0)
    rw = np.concatenate([Cw.T, -Sw.T], axis=1)            # [128(w),130]
    Ch = np.cos(th * np.outer(j, j)) / np.sqrt(128.0)
    Sh = np.sin(th * np.outer(j, j)) / np.sqrt(128.0)
    rh1 = np.concatenate([Ch, -Sh], axis=1)               # [128(h),256] pairs XR
    rh2 = np.concatenate([Sh, Ch], axis=1)                # pairs XI
    rm1 = np.concatenate(
        [np.concatenate([w2r.T, w2i.T], axis=1),
         np.concatenate([b2[:, 0], b2[:, 1]])[None, :]], axis=0)   # [97,192]
    rm2 = np.concatenate(
        [np.concatenate([-w2i.T, w2r.T], axis=1),
         np.zeros((1, 192), np.float32)], axis=0)
    g1 = np.concatenate([Ch, Sh], axis=1)                 # [128(g),256] pairs YR
    g2 = np.concatenate([-Sh, Ch], axis=1)                # pairs YI
    cf = np.full(F, 2.0)
    cf[0] = 1.0
    cf[64] = 1.0
    art = (cf[None, :] * np.cos(th * np.outer(j, f)) / np.sqrt(128.0)).T  # [65,128]
    ait = (-cf[None, :] * np.sin(th * np.outer(j, f)) / np.sqrt(128.0)).T
    c16 = lambda a: np.ascontiguousarray(a).astype(bf)
    return {
        "rw": c16(rw), "rh1": c16(rh1), "rh2": c16(rh2),
        "w1rt": c16(w1r.T), "w1it": c16(w1i.T), "nw1it": c16(-w1i.T),
        "rm1": c16(rm1), "rm2": c16(rm2),
        "g1": c16(g1), "g2": c16(g2), "art": c16(art), "ait": c16(ait),
        "b1r": np.ascontiguousarray(b1[:, 0:1]).astype(np.float32),
        "b1i": np.ascontiguousarray(b1[:, 1:2]).astype(np.float32),
    }


def _build_kernel(ctx, tc, dram):
    import concourse.mybir as mybir

    nc = tc.nc
    bf = mybir.dt.bfloat16
    f32 = mybir.dt.float32
    AF = mybir.ActivationFunctionType
    OP = mybir.AluOpType

    xr = dram["xbf"].ap().rearrange("b h w d -> b w h d")     # [4,128(w),128(h),96]
    outr = dram["out"].ap().rearrange("b h w d -> b w h d")

    consts = ctx.enter_context(tc.tile_pool(name="consts", bufs=1))
    xin = ctx.enter_context(tc.tile_pool(name="xin", bufs=2))
    stg = ctx.enter_context(tc.tile_pool(name="stg", bufs=1))
    hpool = ctx.enter_context(tc.tile_pool(name="hpool", bufs=1))
    apool = ctx.enter_context(tc.tile_pool(name="apool", bufs=2))
    opool = ctx.enter_context(tc.tile_pool(name="opool", bufs=1))
    pp = ctx.enter_context(tc.tile_pool(name="ps", bufs=2, space="PSUM"))

    def cload(name, shape, dtype=bf):
        t = consts.tile(shape, dtype, tag=name)
        nc.sync.dma_start(out=t[:], in_=dram[name].ap())
        return t

    RW = cload("rw", [128, 130])
    RH1 = cload("rh1", [128, 256])
    RH2 = cload("rh2", [128, 256])
    W1RT = cload("w1rt", [96, 96])
    W1IT = cload("w1it", [96, 96])
    NW1IT = cload("nw1it", [96, 96])
    RM1 = cload("rm1", [97, 192])
    RM2 = cload("rm2", [97, 192])
    G1 = cload("g1", [128, 256])
    G2 = cload("g2", [128, 256])
    ART = cload("art", [65, 128])
    AIT = cload("ait", [65, 128])
    B1R = cload("b1r", [96, 1], f32)
    B1I = cload("b1i", [96, 1], f32)

    # persistent MLP hidden tiles with the bias ones-row (row 96)
    HRe = hpool.tile([97, F, 128], bf, tag="hre")
    HIe = hpool.tile([97, F, 128], bf, tag="hie")
    nc.vector.memset(HRe[96:97, :, :], 1.0)
    nc.vector.memset(HIe[96:97, :, :], 0.0)
    NTH = consts.tile([128, 1], f32, tag="nth")   # softshrink -t bias column
    nc.vector.memset(NTH[:, :], -TH)

    # weighted ACT/DVE load balancing for PSUM->SBUF evictions
    eng_ns = {"act": 0.0, "dve": 0.0}

    def evict(dst, src, fd):
        act_cost = (fd + 344) / 1.2
        dve_cost = (fd + 240) / 0.96
        if eng_ns["act"] + act_cost <= eng_ns["dve"] + dve_cost:
            eng_ns["act"] += act_cost
            nc.scalar.activation(out=dst, in_=src, func=AF.Copy)
        else:
            eng_ns["dve"] += dve_cost
            nc.vector.tensor_copy(out=dst, in_=src)

    for b in range(B):
        X0 = xin.tile([128, H, BS], bf, tag="x0")       # [w,(h,d)]
        for hc in range(4):
            nc.sync.dma_start(out=X0[:, hc * 32:(hc + 1) * 32, :],
                              in_=xr[b, :, hc * 32:(hc + 1) * 32, :])
        X0f = X0[:, :, :].rearrange("p h d -> p (h d)")

        # ---- S1: rfft along W.  per d: psum[h,130] = X0[:,:,d].T @ RW
        S1o = stg.tile([128, BS, 130], bf, tag="A")      # [h,(d,{fr|fi})]
        for grp in range(8):                             # 12 d per psum tile
            ps = pp.tile([128, 4, 512], f32, tag="ps")
            for jb in range(4):
                for k in range(3):
                    d = grp * 12 + jb * 3 + k
                    nc.tensor.matmul(ps[:, jb, k * 130:(k + 1) * 130],
                                     X0[:, :, d], RW[:, :],
                                     start=True, stop=True)
            evict(S1o[:, grp * 12:(grp + 1) * 12, :].rearrange(
                      "p (jb k) c -> p jb (k c)", jb=4),
                  ps[:, :, 0:390], 1560)

        # ---- S2: full fft along H. per f: psum[d,256] = XR_f.T@RH1 + XI_f.T@RH2
        ZR = stg.tile([96, F, 128], bf, tag="C")         # [d,(f,g)]
        ZI = stg.tile([96, F, 128], bf, tag="B")
        for grp in range(9):                             # 8 f per psum tile
            nf = min(8, F - grp * 8)
            ps = pp.tile([128, 4, 512], f32, tag="ps")
            for k in range(nf):
                f = grp * 8 + k
                sl = ps[0:96, k // 2, (k % 2) * 256:(k % 2) * 256 + 256]
                nc.tensor.matmul(sl, S1o[:, :, f], RH1[:, :],
                                 start=True, stop=False)
                nc.tensor.matmul(sl, S1o[:, :, 65 + f], RH2[:, :],
                                 start=False, stop=True)
            nbank = (nf + 1) // 2
            kin = min(2, nf)
            src = ps[0:96, :, :].rearrange("p jb (k g) -> p jb k g", k=2)
            dstR = ZR[:, grp * 8:grp * 8 + nf, :].rearrange(
                "p (jb k) g -> p jb k g", k=kin)
            dstI = ZI[:, grp * 8:grp * 8 + nf, :].rearrange(
                "p (jb k) g -> p jb k g", k=kin)
            evict(dstR, src[:, 0:nbank, 0:kin, 0:128], nf * 128)
            evict(dstI, src[:, 0:nbank, 0:kin, 128:256], nf * 128)

        # ---- MLP1 (+bias +relu): contract d. psum[o,cw] over col chunks
        ZRf = ZR[:, :, :].rearrange("p f g -> p (f g)")
        ZIf = ZI[:, :, :].rearrange("p f g -> p (f g)")
        HRf = HRe[0:96, :, :].rearrange("p f g -> p (f g)")
        HIf = HIe[0:96, :, :].rearrange("p f g -> p (f g)")
        nchunk = (FG + 511) // 512                        # 17
        for cp in range(0, nchunk, 2):
            ps = pp.tile([128, 4, 512], f32, tag="ps")
            for ci, c in enumerate(range(cp, min(cp + 2, nchunk))):
                c0 = c * 512
                cw = min(512, FG - c0)
                pr = ps[0:96, 2 * ci, 0:cw]
                pi = ps[0:96, 2 * ci + 1, 0:cw]
                nc.tensor.matmul(pr, W1RT[:, :], ZRf[:, c0:c0 + cw],
                                 start=True, stop=False)
                nc.tensor.matmul(pr, NW1IT[:, :], ZIf[:, c0:c0 + cw],
                                 start=False, stop=True)
                nc.tensor.matmul(pi, W1IT[:, :], ZRf[:, c0:c0 + cw],
                                 start=True, stop=False)
                nc.tensor.matmul(pi, W1RT[:, :], ZIf[:, c0:c0 + cw],
                                 start=False, stop=True)
                nc.scalar.activation(out=HRf[:, c0:c0 + cw], in_=pr,
                                     func=AF.Relu, bias=B1R[:, :], scale=1.0)
                eng_ns["act"] += (cw + 344) / 1.2
                nc.vector.tensor_scalar(out=HIf[:, c0:c0 + cw], in0=pi,
                                        scalar1=B1I[:, :], scalar2=0.0,
                                        op0=OP.add, op1=OP.max)
                eng_ns["dve"] += (cw + 240) / 0.96

        # ---- MLP2 (+b2 via ones-row) fused with softshrink:
        #      v = HRe_f.T@RM1 + HIe_f.T@RM2  (PSUM)
        #      a = relu(v - t)   [ACT];   y = min(v + t, a)   [DVE]
        Y = stg.tile([128, F, 192], bf, tag="C")          # [g,(f,{yr|yi})]
        for grp in range(9):
            nf = min(8, F - grp * 8)
            ps = pp.tile([128, 4, 512], f32, tag="ps")
            for k in range(nf):
                f = grp * 8 + k
                sl = ps[:, k // 2, (k % 2) * 256:(k % 2) * 256 + 192]
                nc.tensor.matmul(sl, HRe[:, f, :], RM1[:, :],
                                 start=True, stop=False)
                nc.tensor.matmul(sl, HIe[:, f, :], RM2[:, :],
                                 start=False, stop=True)
            nbank = (nf + 1) // 2
            kin = min(2, nf)
            src = ps[:, 0:nbank, :].rearrange(
                "p jb (k c) -> p jb k c", k=2)[:, :, 0:kin, 0:192]
            As = apool.tile([128, 8, 192], bf, tag="ash")
            adst = As[:, 0:nf, :].rearrange("p (jb k) c -> p jb k c", k=kin)
            nc.scalar.activation(out=adst, in_=src, func=AF.Relu,
                                 bias=NTH[:, :], scale=1.0)
            eng_ns["act"] += (nf * 192 + 344) / 1.2
            nc.vector.scalar_tensor_tensor(
                out=Y[:, grp * 8:grp * 8 + nf, :].rearrange(
                    "p (jb k) c -> p jb k c", k=kin),
                in0=src, scalar=TH, in1=adst, op0=OP.add, op1=OP.min)
            eng_ns["dve"] += (nf * 192 + 240) / 0.96

        # ---- invH: per d: psum[f,256] = YR_d.T@G1 + YI_d.T@G2
        YspR = stg.tile([65, BS, H], bf, tag="B")         # [f,(d,h)]
        YspI = stg.tile([65, BS, H], bf, tag="A")
        for grp in range(12):                             # 8 d per psum tile
            ps = pp.tile([128, 4, 512], f32, tag="ps")
            for k in range(8):
                d = grp * 8 + k
                sl = ps[0:65, k // 2, (k % 2) * 256:(k % 2) * 256 + 256]
                nc.tensor.matmul(sl, Y[:, :, d], G1[:, :],
                                 start=True, stop=False)
                nc.tensor.matmul(sl, Y[:, :, 96 + d], G2[:, :],
                                 start=False, stop=True)
            src = ps[0:65, :, :].rearrange("p jb (k c) -> p jb k c", k=2)
            d0 = grp * 8
            dstR = YspR[:, d0:d0 + 8, :].rearrange("p (j k) h -> p j k h", j=4)
            dstI = YspI[:, d0:d0 + 8, :].rearrange("p (j k) h -> p j k h", j=4)
            evict(dstR, src[:, :, :, 0:128], 1024)
            evict(dstI, src[:, :, :, 128:256], 1024)

        # ---- invW (+skip): psum[w,cw] = ART.T@YspR + AIT.T@YspI ; out += x
        # rhs streams the permuted [f,(h,d)] view so out lands as [w,(h,d)]
        YRp = YspR[:, :, :].rearrange("p d h -> p h d")
        YIp = YspI[:, :, :].rearrange("p d h -> p h d")
        outt = opool.tile([128, H, BS], bf, tag="outt")   # [w,(h,d)]
        outf = outt[:, :, :].rearrange("p h d -> p (h d)")
        for cp in range(8):                               # 4 h-blocks of 4 per tile
            ps = pp.tile([128, 4, 512], f32, tag="ps")
            for j in range(4):
                h0 = (cp * 4 + j) * 4
                nc.tensor.matmul(ps[:, j, 0:384], ART[:, :], YRp[:, h0:h0 + 4, :],
                                 start=True, stop=False)
                nc.tensor.matmul(ps[:, j, 0:384], AIT[:, :], YIp[:, h0:h0 + 4, :],
                                 start=False, stop=True)
            for j in range(0, 4, 2):
                c0 = (cp * 4 + j) * 4 * BS
                nc.vector.tensor_tensor(
                    out=outf[:, c0:c0 + 768].rearrange("p (j c) -> p j c", j=2),
                    in0=ps[:, j:j + 2, 0:384],
                    in1=X0f[:, c0:c0 + 768].rearrange("p (j c) -> p j c", j=2),
                    op=OP.add)
                eng_ns["dve"] += (768 + 240) / 0.96
        for hc in range(4):
            nc.sync.dma_start(out=outr[b, :, hc * 32:(hc + 1) * 32, :],
                              in_=outt[:, hc * 32:(hc + 1) * 32, :])


def _get_compiled():
    if "nc" in _CACHE:
        return _CACHE["nc"]
    import concourse.mybir as mybir
    import concourse.tile as tile
    from concourse import bacc

    nc = bacc.Bacc("TRN2", target_bir_lowering=False, debug=False)
    bf = mybir.dt.bfloat16
    f32 = mybir.dt.float32
    dram = {}
    dram["xbf"] = nc.dram_tensor("xbf", [B, H, W, BS], bf, kind="ExternalInput")
    for name, shape in [("rw", [128, 130]), ("rh1", [128, 256]),
                        ("rh2", [128, 256]), ("w1rt", [96, 96]),
                        ("w1it", [96, 96]), ("nw1it", [96, 96]),
                        ("rm1", [97, 192]), ("rm2", [97, 192]),
                        ("g1", [128, 256]), ("g2", [128, 256]),
                        ("art", [65, 128]), ("ait", [65, 128])]:
        dram[name] = nc.dram_tensor(name, shape, bf, kind="ExternalInput")
    dram["b1r"] = nc.dram_tensor("b1r", [96, 1], f32, kind="ExternalInput")
    dram["b1i"] = nc.dram_tensor("b1i", [96, 1], f32, kind="ExternalInput")
    dram["out"] = nc.dram_tensor("out", [B, H, W, BS], bf, kind="ExternalOutput")

    from contextlib import ExitStack
    with tile.TileContext(nc) as tc:
        with ExitStack() as ctx:
            _build_kernel(ctx, tc, dram)
    nc.compile()
    _CACHE["nc"] = nc
    return nc


LAST_RESULT = None


def kernel(x, w1r, w1i, b1, w2r, w2i, b2):
    global LAST_RESULT
    from concourse.bass_utils import run_bass_kernel_spmd

    x = np.asarray(x)
    consts = _make_consts(np.asarray(w1r, np.float32), np.asarray(w1i, np.float32),
                          np.asarray(b1, np.float32), np.asarray(w2r, np.float32),
                          np.asarray(w2i, np.float32), np.asarray(b2, np.float32))
    nc = _get_compiled()
    in_maps = []
    for c in range(NCORES):
        m = dict(consts)
        m["xbf"] = np.ascontiguousarray(
            x[:, :, :, c * BS:(c + 1) * BS]).astype(ml_dtypes.bfloat16)
        in_maps.append(m)
    res = run_bass_kernel_spmd(nc, in_maps, core_ids=list(range(NCORES)))
    LAST_RESULT = res
    out = np.concatenate(
        [res.results[c]["out"].astype(np.float32) for c in range(NCORES)], axis=3)
    return out


# revision 25
# speedup vs baseline: 1.5327x; 1.1638x over previous
"""AFNO layer (2D rFFT -> block-diag complex MLP -> softshrink -> irFFT -> +skip)
as a Bass/Tile kernel on 8 TRN2 NeuronCores.

Sharding: the num_blocks axis (NB=8 blocks of 96 channels) maps one block per
core -- the FFTs are per-channel over spatial dims and the MLP mixes only
within a block, so the 8 cores are fully independent (no collectives).

All DFTs are dense matmuls against precomputed (host-side) DFT matrices in
bf16; accumulation is fp32 in PSUM.  Every stage is laid out so the tensor
engine contraction dim (SBUF partition dim) chains through the pipeline:

  x[w,(h,d)] --S1(rfft_W)--> [h,(d,f)] --S2(fft_H)--> [d,(f,g)]
    --MLP1--> [o,(f,g)] --MLP2(+b2)--> [g,(f,{vr|vi})] --softshrink-->
    --invH--> [f,(h,d)] --invW(+skip)--> out[w,(h,d)]
"""

import numpy as np
import ml_dtypes

B = 4
H = 128
W = 128
D = 768
BS = 96          # block size = channels per core
F = 65           # rfft bins along W
NCORES = 8
TH = 0.01        # softshrink threshold
FG = F * 128     # positions per (f,g) plane

_CACHE = {}


def _make_consts(w1r, w1i, b1, w2r, w2i, b2):
    """Host-side constant matrices, keyed as the kernel's dram inputs."""
    bf = ml_dtypes.bfloat16
    th = 2 * np.pi / 128
    j = np.arange(128)
    f = np.arange(F)
    Cw = np.cos(th * np.outer(f, j)) / np.sqrt(128.0)
    Sw = np.sin(th * np.outer(f, j)) / np.sqrt(128.0)
    rw = np.concatenate([Cw.T, -Sw.T], axis=1)            # [128(w),130]
    Ch = np.cos(th * np.outer(j, j)) / np.sqrt(128.0)
    Sh = np.sin(th * np.outer(j, j)) / np.sqrt(128.0)
    rh1 = np.concatenate([Ch, -Sh], axis=1)               # [128(h),256] pairs XR
    rh2 = np.concatenate([Sh, Ch], axis=1)                # pairs XI
    rm1 = np.concatenate(
        [np.concatenate([w2r.T, w2i.T], axis=1),
         np.concatenate([b2[:, 0], b2[:, 1]])[None, :]], axis=0)   # [97,192]
    rm2 = np.concatenate(
        [np.concatenate([-w2i.T, w2r.T], axis=1),
         np.zeros((1, 192), np.float32)], axis=0)
    g1 = np.concatenate([Ch, Sh], axis=1)                 # [128(g),256] pairs YR
    g2 = np.concatenate([-Sh, Ch], axis=1)                # pairs YI
    cf = np.full(F, 2.0)
    cf[0] = 1.0
    cf[64] = 1.0
    art = (cf[None, :] * np.cos(th * np.outer(j, f)) / np.sqrt(128.0)).T  # [65,128]
    ait = (-cf[None, :] * np.sin(th * np.outer(j, f)) / np.sqrt(128.0)).T
    c16 = lambda a: np.ascontiguousarray(a).astype(bf)
    return {
        "rw": c16(rw), "rh1": c16(rh1), "rh2": c16(rh2),
        "w1rt": c16(w1r.T), "w1it": c16(w1i.T), "nw1it": c16(-w1i.T),
        "rm1": c16(rm1), "rm2": c16(rm2),
        "g1": c16(g1), "g2": c16(g2), "art": c16(art), "ait": c16(ait),
        "b1r": np.ascontiguousarray(b1[:, 0:1]).astype(np.float32),
        "b1i": np.ascontiguousarray(b1[:, 1:2]).astype(np.float32),
    }


def _build_kernel(ctx, tc, dram):
    import concourse.mybir as mybir

    nc = tc.nc
    bf = mybir.dt.bfloat16
    f32 = mybir.dt.float32
    AF = mybir.ActivationFunctionType
    OP = mybir.AluOpType

    xr = dram["xbf"].ap().rearrange("b h w d -> b w h d")     # [4,128(w),128(h),96]
    outr = dram["out"].ap().rearrange("b h w d -> b w h d")

    consts = ctx.enter_context(tc.tile_pool(name="consts", bufs=1))
    xin = ctx.enter_context(tc.tile_pool(name="xin", bufs=2))
    stg = ctx.enter_context(tc.tile_pool(name="stg", bufs=1))
    hpool = ctx.enter_context(tc.tile_pool(name="hpool", bufs=1))
    apool = ctx.enter_context(tc.tile_pool(name="apool", bufs=2))
    opool = ctx.enter_context(tc.tile_pool(name="opool", bufs=1))
    pp = ctx.enter_context(tc.tile_pool(name="ps", bufs=4, space="PSUM"))

    def cload(name, shape, dtype=bf):
        t = consts.tile(shape, dtype, tag=name)
        nc.sync.dma_start(out=t[:], in_=dram[name].ap())
        return t

    RW = cload("rw", [128, 130])
    RH1 = cload("rh1", [128, 256])
    RH2 = cload("rh2", [128, 256])
    W1RT = cload("w1rt", [96, 96])
    W1IT = cload("w1it", [96, 96])
    NW1IT = cload("nw1it", [96, 96])
    RM1 = cload("rm1", [97, 192])
    RM2 = cload("rm2", [97, 192])
    G1 = cload("g1", [128, 256])
    G2 = cload("g2", [128, 256])
    ART = cload("art", [65, 128])
    AIT = cload("ait", [65, 128])
    B1R = cload("b1r", [96, 1], f32)
    B1I = cload("b1i", [96, 1], f32)

    # persistent MLP hidden tiles with the bias ones-row (row 96)
    HRe = hpool.tile([97, F, 128], bf, tag="hre")
    HIe = hpool.tile([97, F, 128], bf, tag="hie")
    nc.vector.memset(HRe[96:97, :, :], 1.0)
    nc.vector.memset(HIe[96:97, :, :], 0.0)
    NTH = consts.tile([128, 1], f32, tag="nth")   # softshrink -t bias column
    nc.vector.memset(NTH[:, :], -TH)

    # weighted ACT/DVE load balancing for PSUM->SBUF evictions
    eng_ns = {"act": 0.0, "dve": 0.0}

    def evict(dst, src, fd):
        act_cost = (fd + 344) / 1.2
        dve_cost = (fd + 240) / 0.96
        if eng_ns["act"] + act_cost <= eng_ns["dve"] + dve_cost:
            eng_ns["act"] += act_cost
            nc.scalar.activation(out=dst, in_=src, func=AF.Copy)
        else:
            eng_ns["dve"] += dve_cost
            nc.vector.tensor_copy(out=dst, in_=src)

    for b in range(B):
        X0 = xin.tile([128, H, BS], bf, tag="x0")       # [w,(h,d)]
        for hc in range(4):
            nc.sync.dma_start(out=X0[:, hc * 32:(hc + 1) * 32, :],
                              in_=xr[b, :, hc * 32:(hc + 1) * 32, :])
        X0f = X0[:, :, :].rearrange("p h d -> p (h d)")

        # ---- S1: rfft along W.  per d: psum[h,130] = X0[:,:,d].T @ RW
        S1o = stg.tile([128, BS, 130], bf, tag="A")      # [h,(d,{fr|fi})]
        for grp in range(16):                            # 6 d per psum tile
            ps = pp.tile([128, 2, 512], f32, tag="ps")
            for jb in range(2):
                for k in range(3):
                    d = grp * 6 + jb * 3 + k
                    nc.tensor.matmul(ps[:, jb, k * 130:(k + 1) * 130],
                                     X0[:, :, d], RW[:, :],
                                     start=True, stop=True)
            evict(S1o[:, grp * 6:(grp + 1) * 6, :].rearrange(
                      "p (jb k) c -> p jb (k c)", jb=2),
                  ps[:, :, 0:390], 780)

        # ---- S2: full fft along H. per f: psum[d,256] = XR_f.T@RH1 + XI_f.T@RH2
        ZR = stg.tile([96, F, 128], bf, tag="C")         # [d,(f,g)]
        ZI = stg.tile([96, F, 128], bf, tag="B")
        for grp in range(17):                            # 4 f per psum tile
            nf = min(4, F - grp * 4)
            ps = pp.tile([128, 2, 512], f32, tag="ps")
            for k in range(nf):
                f = grp * 4 + k
                sl = ps[0:96, k // 2, (k % 2) * 256:(k % 2) * 256 + 256]
                nc.tensor.matmul(sl, S1o[:, :, f], RH1[:, :],
                                 start=True, stop=False)
                nc.tensor.matmul(sl, S1o[:, :, 65 + f], RH2[:, :],
                                 start=False, stop=True)
            nbank = (nf + 1) // 2
            kin = min(2, nf)
            src = ps[0:96, :, :].rearrange("p jb (k g) -> p jb k g", k=2)
            dstR = ZR[:, grp * 4:grp * 4 + nf, :].rearrange(
                "p (jb k) g -> p jb k g", k=kin)
            dstI = ZI[:, grp * 4:grp * 4 + nf, :].rearrange(
                "p (jb k) g -> p jb k g", k=kin)
            evict(dstR, src[:, 0:nbank, 0:kin, 0:128], nf * 128)
            evict(dstI, src[:, 0:nbank, 0:kin, 128:256], nf * 128)

        # ---- MLP1 (+bias +relu): contract d. psum[o,cw] over col chunks
        ZRf = ZR[:, :, :].rearrange("p f g -> p (f g)")
        ZIf = ZI[:, :, :].rearrange("p f g -> p (f g)")
        HRf = HRe[0:96, :, :].rearrange("p f g -> p (f g)")
        HIf = HIe[0:96, :, :].rearrange("p f g -> p (f g)")
        nchunk = (FG + 511) // 512                        # 17
        for c in range(nchunk):
            ps = pp.tile([128, 2, 512], f32, tag="ps")
            if True:
                c0 = c * 512
                cw = min(512, FG - c0)
                pr = ps[0:96, 0, 0:cw]
                pi = ps[0:96, 1, 0:cw]
                nc.tensor.matmul(pr, W1RT[:, :], ZRf[:, c0:c0 + cw],
                                 start=True, stop=False)
                nc.tensor.matmul(pr, NW1IT[:, :], ZIf[:, c0:c0 + cw],
                                 start=False, stop=True)
                nc.tensor.matmul(pi, W1IT[:, :], ZRf[:, c0:c0 + cw],
                                 start=True, stop=False)
                nc.tensor.matmul(pi, W1RT[:, :], ZIf[:, c0:c0 + cw],
                                 start=False, stop=True)
                nc.scalar.activation(out=HRf[:, c0:c0 + cw], in_=pr,
                                     func=AF.Relu, bias=B1R[:, :], scale=1.0)
                eng_ns["act"] += (cw + 344) / 1.2
                nc.vector.tensor_scalar(out=HIf[:, c0:c0 + cw], in0=pi,
                                        scalar1=B1I[:, :], scalar2=0.0,
                                        op0=OP.add, op1=OP.max)
                eng_ns["dve"] += (cw + 240) / 0.96

        # ---- MLP2 (+b2 via ones-row) fused with softshrink:
        #      v = HRe_f.T@RM1 + HIe_f.T@RM2  (PSUM)
        #      a = relu(v - t)   [ACT];   y = min(v + t, a)   [DVE]
        Y = stg.tile([128, F, 192], bf, tag="C")          # [g,(f,{yr|yi})]
        for grp in range(17):
            nf = min(4, F - grp * 4)
            ps = pp.tile([128, 2, 512], f32, tag="ps")
            for k in range(nf):
                f = grp * 4 + k
                sl = ps[:, k // 2, (k % 2) * 256:(k % 2) * 256 + 192]
                nc.tensor.matmul(sl, HRe[:, f, :], RM1[:, :],
                                 start=True, stop=False)
                nc.tensor.matmul(sl, HIe[:, f, :], RM2[:, :],
                                 start=False, stop=True)
            nbank = (nf + 1) // 2
            kin = min(2, nf)
            src = ps[:, 0:nbank, :].rearrange(
                "p jb (k c) -> p jb k c", k=2)[:, :, 0:kin, 0:192]
            As = apool.tile([128, 4, 192], bf, tag="ash")
            adst = As[:, 0:nf, :].rearrange("p (jb k) c -> p jb k c", k=kin)
            nc.scalar.activation(out=adst, in_=src, func=AF.Relu,
                                 bias=NTH[:, :], scale=1.0)
            eng_ns["act"] += (nf * 192 + 344) / 1.2
            nc.vector.scalar_tensor_tensor(
                out=Y[:, grp * 4:grp * 4 + nf, :].rearrange(
                    "p (jb k) c -> p jb k c", k=kin),
                in0=src, scalar=TH, in1=adst, op0=OP.add, op1=OP.min)
            eng_ns["dve"] += (nf * 192 + 240) / 0.96

        # ---- invH: per d: psum[f,256] = YR_d.T@G1 + YI_d.T@G2
        YspR = stg.tile([65, BS, H], bf, tag="B")         # [f,(d,h)]
        YspI = stg.tile([65, BS, H], bf, tag="A")
        for grp in range(24):                             # 4 d per psum tile
            ps = pp.tile([128, 2, 512], f32, tag="ps")
            for k in range(4):
                d = grp * 4 + k
                sl = ps[0:65, k // 2, (k % 2) * 256:(k % 2) * 256 + 256]
                nc.tensor.matmul(sl, Y[:, :, d], G1[:, :],
                                 start=True, stop=False)
                nc.tensor.matmul(sl, Y[:, :, 96 + d], G2[:, :],
                                 start=False, stop=True)
            src = ps[0:65, :, :].rearrange("p jb (k c) -> p jb k c", k=2)
            d0 = grp * 4
            dstR = YspR[:, d0:d0 + 4, :].rearrange("p (j k) h -> p j k h", j=2)
            dstI = YspI[:, d0:d0 + 4, :].rearrange("p (j k) h -> p j k h", j=2)
            evict(dstR, src[:, :, :, 0:128], 512)
            evict(dstI, src[:, :, :, 128:256], 512)

        # ---- invW (+skip): psum[w,cw] = ART.T@YspR + AIT.T@YspI ; out += x
        # rhs streams the permuted [f,(h,d)] view so out lands as [w,(h,d)]
        YRp = YspR[:, :, :].rearrange("p d h -> p h d")
        YIp = YspI[:, :, :].rearrange("p d h -> p h d")
        outt = opool.tile([128, H, BS], bf, tag="outt")   # [w,(h,d)]
        outf = outt[:, :, :].rearrange("p h d -> p (h d)")
        for cp in range(16):                              # 2 h-blocks of 4 per tile
            ps = pp.tile([128, 2, 512], f32, tag="ps")
            for j in range(2):
                h0 = (cp * 2 + j) * 4
                nc.tensor.matmul(ps[:, j, 0:384], ART[:, :], YRp[:, h0:h0 + 4, :],
                                 start=True, stop=False)
                nc.tensor.matmul(ps[:, j, 0:384], AIT[:, :], YIp[:, h0:h0 + 4, :],
                                 start=False, stop=True)
            c0 = cp * 2 * 4 * BS
            nc.vector.tensor_tensor(
                out=outf[:, c0:c0 + 768].rearrange("p (j c) -> p j c", j=2),
                in0=ps[:, 0:2, 0:384],
                in1=X0f[:, c0:c0 + 768].rearrange("p (j c) -> p j c", j=2),
                op=OP.add)
            eng_ns["dve"] += (768 + 240) / 0.96
        for hc in range(4):
            nc.sync.dma_start(out=outr[b, :, hc * 32:(hc + 1) * 32, :],
                              in_=outt[:, hc * 32:(hc + 1) * 32, :])


def _get_compiled():
    if "nc" in _CACHE:
        return _CACHE["nc"]
    import concourse.mybir as mybir
    import concourse.tile as tile
    from concourse import bacc

    nc = bacc.Bacc("TRN2", target_bir_lowering=False, debug=False)
    bf = mybir.dt.bfloat16
    f32 = mybir.dt.float32
    dram = {}
    dram["xbf"] = nc.dram_tensor("xbf", [B, H, W, BS], bf, kind="ExternalInput")
    for name, shape in [("rw", [128, 130]), ("rh1", [128, 256]),
                        ("rh2", [128, 256]), ("w1rt", [96, 96]),
                        ("w1it", [96, 96]), ("nw1it", [96, 96]),
                        ("rm1", [97, 192]), ("rm2", [97, 192]),
                        ("g1", [128, 256]), ("g2", [128, 256]),
                        ("art", [65, 128]), ("ait", [65, 128])]:
        dram[name] = nc.dram_tensor(name, shape, bf, kind="ExternalInput")
    dram["b1r"] = nc.dram_tensor("b1r", [96, 1], f32, kind="ExternalInput")
    dram["b1i"] = nc.dram_tensor("b1i", [96, 1], f32, kind="ExternalInput")
    dram["out"] = nc.dram_tensor("out", [B, H, W, BS], bf, kind="ExternalOutput")

    from contextlib import ExitStack
    with tile.TileContext(nc) as tc:
        with ExitStack() as ctx:
            _build_kernel(ctx, tc, dram)
    nc.compile()
    _CACHE["nc"] = nc
    return nc


LAST_RESULT = None


def kernel(x, w1r, w1i, b1, w2r, w2i, b2):
    global LAST_RESULT
    from concourse.bass_utils import run_bass_kernel_spmd

    x = np.asarray(x)
    consts = _make_consts(np.asarray(w1r, np.float32), np.asarray(w1i, np.float32),
                          np.asarray(b1, np.float32), np.asarray(w2r, np.float32),
                          np.asarray(w2i, np.float32), np.asarray(b2, np.float32))
    nc = _get_compiled()
    in_maps = []
    for c in range(NCORES):
        m = dict(consts)
        m["xbf"] = np.ascontiguousarray(
            x[:, :, :, c * BS:(c + 1) * BS]).astype(ml_dtypes.bfloat16)
        in_maps.append(m)
    res = run_bass_kernel_spmd(nc, in_maps, core_ids=list(range(NCORES)))
    LAST_RESULT = res
    out = np.concatenate(
        [res.results[c]["out"].astype(np.float32) for c in range(NCORES)], axis=3)
    return out


# revision 30
# speedup vs baseline: 1.6226x; 1.0586x over previous
"""AFNO layer (2D rFFT -> block-diag complex MLP -> softshrink -> irFFT -> +skip)
as a Bass/Tile kernel on 8 TRN2 NeuronCores.

Sharding: the num_blocks axis (NB=8 blocks of 96 channels) maps one block per
core -- the FFTs are per-channel over spatial dims and the MLP mixes only
within a block, so the 8 cores are fully independent (no collectives).

All DFTs are dense matmuls against precomputed (host-side) DFT matrices in
bf16; accumulation is fp32 in PSUM.  Every stage is laid out so the tensor
engine contraction dim (SBUF partition dim) chains through the pipeline:

  x[w,(h,d)] --S1(rfft_W)--> [h,(d,f)] --S2(fft_H)--> [d,(f,g)]
    --MLP1--> [o,(f,g)] --MLP2(+b2)--> [g,(f,{vr|vi})] --softshrink-->
    --invH--> [f,(h,d)] --invW(+skip)--> out[w,(h,d)]
"""

import numpy as np
import ml_dtypes

B = 4
H = 128
W = 128
D = 768
BS = 96          # block size = channels per core
F = 65           # rfft bins along W
NCORES = 8
TH = 0.01        # softshrink threshold
FG = F * 128     # positions per (f,g) plane

_CACHE = {}


def _make_consts(w1r, w1i, b1, w2r, w2i, b2):
    """Host-side constant matrices, keyed as the kernel's dram inputs."""
    bf = ml_dtypes.bfloat16
    th = 2 * np.pi / 128
    j = np.arange(128)
    f = np.arange(F)
    Cw = np.cos(th * np.outer(f, j)) / np.sqrt(128.0)
    Sw = np.sin(th * np.outer(f, j)) / np.sqrt(128.0)
    rw = np.concatenate([Cw.T, -Sw.T], axis=1)            # [128(w),130]
    Ch = np.cos(th * np.outer(j, j)) / np.sqrt(128.0)
    Sh = np.sin(th * np.outer(j, j)) / np.sqrt(128.0)
    rh1 = np.concatenate([Ch, -Sh], axis=1)               # [128(h),256] pairs XR
    rh2 = np.concatenate([Sh, Ch], axis=1)                # pairs XI
    rm1 = np.concatenate(
        [np.concatenate([w2r.T, w2i.T], axis=1),
         np.concatenate([b2[:, 0], b2[:, 1]])[None, :]], axis=0)   # [97,192]
    rm2 = np.concatenate(
        [np.concatenate([-w2i.T, w2r.T], axis=1),
         np.zeros((1, 192), np.float32)], axis=0)
    g1 = np.concatenate([Ch, Sh], axis=1)                 # [128(g),256] pairs YR
    g2 = np.concatenate([-Sh, Ch], axis=1)                # pairs YI
    cf = np.full(F, 2.0)
    cf[0] = 1.0
    cf[64] = 1.0
    art = (cf[None, :] * np.cos(th * np.outer(j, f)) / np.sqrt(128.0)).T  # [65,128]
    ait = (-cf[None, :] * np.sin(th * np.outer(j, f)) / np.sqrt(128.0)).T
    c16 = lambda a: np.ascontiguousarray(a).astype(bf)
    return {
        "rw": c16(rw), "rh1": c16(rh1), "rh2": c16(rh2),
        "w1rt": c16(w1r.T), "w1it": c16(w1i.T), "nw1it": c16(-w1i.T),
        "rm1": c16(rm1), "rm2": c16(rm2),
        "g1": c16(g1), "g2": c16(g2), "art": c16(art), "ait": c16(ait),
        "b1r": np.ascontiguousarray(b1[:, 0:1]).astype(np.float32),
        "b1i": np.ascontiguousarray(b1[:, 1:2]).astype(np.float32),
    }


def _build_kernel(ctx, tc, dram):
    import concourse.mybir as mybir

    nc = tc.nc
    bf = mybir.dt.bfloat16
    f32 = mybir.dt.float32
    AF = mybir.ActivationFunctionType
    OP = mybir.AluOpType

    xr = dram["xbf"].ap().rearrange("b h w d -> b w h d")     # [4,128(w),128(h),96]
    outr = dram["out"].ap().rearrange("b h w d -> b w h d")

    consts = ctx.enter_context(tc.tile_pool(name="consts", bufs=1))
    xin = ctx.enter_context(tc.tile_pool(name="xin", bufs=1))
    stg = ctx.enter_context(tc.tile_pool(name="stg", bufs=1))
    hpool = ctx.enter_context(tc.tile_pool(name="hpool", bufs=1))
    apool = ctx.enter_context(tc.tile_pool(name="apool", bufs=2))
    opool = ctx.enter_context(tc.tile_pool(name="opool", bufs=2))
    skp = ctx.enter_context(tc.tile_pool(name="skp", bufs=3))
    pp = ctx.enter_context(tc.tile_pool(name="ps", bufs=4, space="PSUM"))

    def cload(name, shape, dtype=bf):
        t = consts.tile(shape, dtype, tag=name)
        nc.sync.dma_start(out=t[:], in_=dram[name].ap())
        return t

    RW = cload("rw", [128, 130])
    RH1 = cload("rh1", [128, 256])
    RH2 = cload("rh2", [128, 256])
    W1RT = cload("w1rt", [96, 96])
    W1IT = cload("w1it", [96, 96])
    NW1IT = cload("nw1it", [96, 96])
    RM1 = cload("rm1", [97, 192])
    RM2 = cload("rm2", [97, 192])
    G1 = cload("g1", [128, 256])
    G2 = cload("g2", [128, 256])
    ART = cload("art", [65, 128])
    AIT = cload("ait", [65, 128])
    B1R = cload("b1r", [96, 1], f32)
    B1I = cload("b1i", [96, 1], f32)

    # persistent MLP hidden tiles with the bias ones-row (row 96)
    HRe = hpool.tile([97, F, 128], bf, tag="hre")
    HIe = hpool.tile([97, F, 128], bf, tag="hie")
    nc.vector.memset(HRe[96:97, :, :], 1.0)
    nc.vector.memset(HIe[96:97, :, :], 0.0)
    NTH = consts.tile([128, 1], f32, tag="nth")   # softshrink -t bias column
    nc.vector.memset(NTH[:, :], -TH)

    # weighted ACT/DVE load balancing for PSUM->SBUF evictions
    eng_ns = {"act": 0.0, "dve": 0.0}

    def evict(dst, src, fd):
        act_cost = (fd + 344) / 1.2
        dve_cost = (fd + 240) / 0.96
        if eng_ns["act"] + act_cost <= eng_ns["dve"] + dve_cost:
            eng_ns["act"] += act_cost
            nc.scalar.activation(out=dst, in_=src, func=AF.Copy)
        else:
            eng_ns["dve"] += dve_cost
            nc.vector.tensor_copy(out=dst, in_=src)

    def emit_s1(b):
        """Load x[b] and run the W-rfft; returns the S1 output tile."""
        X0 = xin.tile([128, H, BS], bf, tag="x0")       # [w,(h,d)]
        for hc in range(4):
            nc.sync.dma_start(out=X0[:, hc * 32:(hc + 1) * 32, :],
                              in_=xr[b, :, hc * 32:(hc + 1) * 32, :])
        S1o = stg.tile([128, BS, 130], bf, tag="S")      # [h,(d,{fr|fi})]
        for grp in range(16):                            # 6 d per psum tile
            ps = pp.tile([128, 2, 512], f32, tag="ps")
            for jb in range(2):
                for k in range(3):
                    d = grp * 6 + jb * 3 + k
                    nc.tensor.matmul(ps[:, jb, k * 130:(k + 1) * 130],
                                     X0[:, :, d], RW[:, :],
                                     start=True, stop=True)
            evict(S1o[:, grp * 6:(grp + 1) * 6, :].rearrange(
                      "p (jb k) c -> p jb (k c)", jb=2),
                  ps[:, :, 0:390], 780)
        return S1o

    S1o = emit_s1(0)
    for b in range(B):
        # ---- S2: full fft along H. per f: psum[d,256] = XR_f.T@RH1 + XI_f.T@RH2
        ZR = stg.tile([96, F, 128], bf, tag="C")         # [d,(f,g)]
        ZI = stg.tile([96, F, 128], bf, tag="B")
        for grp in range(17):                            # 4 f per psum tile
            nf = min(4, F - grp * 4)
            ps = pp.tile([128, 2, 512], f32, tag="ps")
            for k in range(nf):
                f = grp * 4 + k
                sl = ps[0:96, k // 2, (k % 2) * 256:(k % 2) * 256 + 256]
                nc.tensor.matmul(sl, S1o[:, :, f], RH1[:, :],
                                 start=True, stop=False)
                nc.tensor.matmul(sl, S1o[:, :, 65 + f], RH2[:, :],
                                 start=False, stop=True)
            nbank = (nf + 1) // 2
            kin = min(2, nf)
            src = ps[0:96, :, :].rearrange("p jb (k g) -> p jb k g", k=2)
            dstR = ZR[:, grp * 4:grp * 4 + nf, :].rearrange(
                "p (jb k) g -> p jb k g", k=kin)
            dstI = ZI[:, grp * 4:grp * 4 + nf, :].rearrange(
                "p (jb k) g -> p jb k g", k=kin)
            evict(dstR, src[:, 0:nbank, 0:kin, 0:128], nf * 128)
            evict(dstI, src[:, 0:nbank, 0:kin, 128:256], nf * 128)

        # ---- MLP1 (+bias +relu): contract d. psum[o,cw] over col chunks
        ZRf = ZR[:, :, :].rearrange("p f g -> p (f g)")
        ZIf = ZI[:, :, :].rearrange("p f g -> p (f g)")
        HRf = HRe[0:96, :, :].rearrange("p f g -> p (f g)")
        HIf = HIe[0:96, :, :].rearrange("p f g -> p (f g)")
        nchunk = (FG + 511) // 512                        # 17
        for c in range(nchunk):
            ps = pp.tile([128, 2, 512], f32, tag="ps")
            if True:
                c0 = c * 512
                cw = min(512, FG - c0)
                pr = ps[0:96, 0, 0:cw]
                pi = ps[0:96, 1, 0:cw]
                nc.tensor.matmul(pr, W1RT[:, :], ZRf[:, c0:c0 + cw],
                                 start=True, stop=False)
                nc.tensor.matmul(pr, NW1IT[:, :], ZIf[:, c0:c0 + cw],
                                 start=False, stop=True)
                nc.tensor.matmul(pi, W1IT[:, :], ZRf[:, c0:c0 + cw],
                                 start=True, stop=False)
                nc.tensor.matmul(pi, W1RT[:, :], ZIf[:, c0:c0 + cw],
                                 start=False, stop=True)
                nc.scalar.activation(out=HRf[:, c0:c0 + cw], in_=pr,
                                     func=AF.Relu, bias=B1R[:, :], scale=1.0)
                eng_ns["act"] += (cw + 344) / 1.2
                nc.vector.tensor_scalar(out=HIf[:, c0:c0 + cw], in0=pi,
                                        scalar1=B1I[:, :], scalar2=0.0,
                                        op0=OP.add, op1=OP.max)
                eng_ns["dve"] += (cw + 240) / 0.96

        # ---- MLP2 (+b2 via ones-row) fused with softshrink:
        #      v = HRe_f.T@RM1 + HIe_f.T@RM2  (PSUM)
        #      a = relu(v - t)   [ACT];   y = min(v + t, a)   [DVE]
        Y = stg.tile([128, F, 192], bf, tag="C")          # [g,(f,{yr|yi})]
        for grp in range(17):
            nf = min(4, F - grp * 4)
            ps = pp.tile([128, 2, 512], f32, tag="ps")
            for k in range(nf):
                f = grp * 4 + k
                sl = ps[:, k // 2, (k % 2) * 256:(k % 2) * 256 + 192]
                nc.tensor.matmul(sl, HRe[:, f, :], RM1[:, :],
                                 start=True, stop=False)
                nc.tensor.matmul(sl, HIe[:, f, :], RM2[:, :],
                                 start=False, stop=True)
            nbank = (nf + 1) // 2
            kin = min(2, nf)
            src = ps[:, 0:nbank, :].rearrange(
                "p jb (k c) -> p jb k c", k=2)[:, :, 0:kin, 0:192]
            As = apool.tile([128, 4, 192], bf, tag="ash")
            adst = As[:, 0:nf, :].rearrange("p (jb k) c -> p jb k c", k=kin)
            nc.scalar.activation(out=adst, in_=src, func=AF.Relu,
                                 bias=NTH[:, :], scale=1.0)
            eng_ns["act"] += (nf * 192 + 344) / 1.2
            nc.vector.scalar_tensor_tensor(
                out=Y[:, grp * 4:grp * 4 + nf, :].rearrange(
                    "p (jb k) c -> p jb k c", k=kin),
                in0=src, scalar=TH, in1=adst, op0=OP.add, op1=OP.min)
            eng_ns["dve"] += (nf * 192 + 240) / 0.96

        # ---- software-pipelined S1(b+1): its matmuls fill the PE during the
        #      LDW-bound invH/invW phases below
        S1o_next = emit_s1(b + 1) if b + 1 < B else None

        # ---- invH: per d: psum[f,256] = YR_d.T@G1 + YI_d.T@G2
        YspR = stg.tile([65, BS, H], bf, tag="B")         # [f,(d,h)]
        YspI = stg.tile([65, BS, H], bf, tag="A")
        for grp in range(24):                             # 4 d per psum tile
            ps = pp.tile([128, 2, 512], f32, tag="ps")
            for k in range(4):
                d = grp * 4 + k
                sl = ps[0:65, k // 2, (k % 2) * 256:(k % 2) * 256 + 256]
                nc.tensor.matmul(sl, Y[:, :, d], G1[:, :],
                                 start=True, stop=False)
                nc.tensor.matmul(sl, Y[:, :, 96 + d], G2[:, :],
                                 start=False, stop=True)
            src = ps[0:65, :, :].rearrange("p jb (k c) -> p jb k c", k=2)
            d0 = grp * 4
            dstR = YspR[:, d0:d0 + 4, :].rearrange("p (j k) h -> p j k h", j=2)
            dstI = YspI[:, d0:d0 + 4, :].rearrange("p (j k) h -> p j k h", j=2)
            evict(dstR, src[:, :, :, 0:128], 512)
            evict(dstI, src[:, :, :, 128:256], 512)

        # ---- invW (+skip): psum[w,cw] = ART.T@YspR + AIT.T@YspI ; out += x
        # rhs streams the permuted [f,(h,d)] view so out lands as [w,(h,d)]
        YRp = YspR[:, :, :].rearrange("p d h -> p h d")
        YIp = YspI[:, :, :].rearrange("p d h -> p h d")
        for cp in range(16):                              # 2 h-blocks of 4 per tile
            h0 = cp * 8
            if cp % 4 == 0:                               # 32 h rows per out tile
                outt = opool.tile([128, 32, BS], bf, tag="outt")
            sk = skp.tile([128, 8, BS], bf, tag="skp")    # skip x re-load
            nc.gpsimd.dma_start(out=sk[:, :, :], in_=xr[b, :, h0:h0 + 8, :])
            ps = pp.tile([128, 2, 512], f32, tag="ps")
            for j in range(2):
                nc.tensor.matmul(ps[:, j, 0:384], ART[:, :],
                                 YRp[:, h0 + 4 * j:h0 + 4 * j + 4, :],
                                 start=True, stop=False)
                nc.tensor.matmul(ps[:, j, 0:384], AIT[:, :],
                                 YIp[:, h0 + 4 * j:h0 + 4 * j + 4, :],
                                 start=False, stop=True)
            nc.vector.tensor_tensor(
                out=outt[:, (cp % 4) * 8:(cp % 4) * 8 + 8, :].rearrange(
                    "p (j hh) d -> p j (hh d)", j=2),
                in0=ps[:, 0:2, 0:384],
                in1=sk[:, :, :].rearrange("p (j hh) d -> p j (hh d)", j=2),
                op=OP.add)
            eng_ns["dve"] += (768 + 240) / 0.96
            if cp % 4 == 3:
                hc = cp // 4
                nc.sync.dma_start(out=outr[b, :, hc * 32:(hc + 1) * 32, :],
                                  in_=outt[:, :, :])
        S1o = S1o_next


def _get_compiled():
    if "nc" in _CACHE:
        return _CACHE["nc"]
    import concourse.mybir as mybir
    import concourse.tile as tile
    from concourse import bacc

    nc = bacc.Bacc("TRN2", target_bir_lowering=False, debug=False)
    bf = mybir.dt.bfloat16
    f32 = mybir.dt.float32
    dram = {}
    dram["xbf"] = nc.dram_tensor("xbf", [B, H, W, BS], bf, kind="ExternalInput")
    for name, shape in [("rw", [128, 130]), ("rh1", [128, 256]),
                        ("rh2", [128, 256]), ("w1rt", [96, 96]),
                        ("w1it", [96, 96]), ("nw1it", [96, 96]),
                        ("rm1", [97, 192]), ("rm2", [97, 192]),
                        ("g1", [128, 256]), ("g2", [128, 256]),
                        ("art", [65, 128]), ("ait", [65, 128])]:
        dram[name] = nc.dram_tensor(name, shape, bf, kind="ExternalInput")
    dram["b1r"] = nc.dram_tensor("b1r", [96, 1], f32, kind="ExternalInput")
    dram["b1i"] = nc.dram_tensor("b1i", [96, 1], f32, kind="ExternalInput")
    dram["out"] = nc.dram_tensor("out", [B, H, W, BS], bf, kind="ExternalOutput")

    from contextlib import ExitStack
    with tile.TileContext(nc) as tc:
        with ExitStack() as ctx:
            _build_kernel(ctx, tc, dram)
    nc.compile()
    _CACHE["nc"] = nc
    return nc


LAST_RESULT = None


def kernel(x, w1r, w1i, b1, w2r, w2i, b2):
    global LAST_RESULT
    from concourse.bass_utils import run_bass_kernel_spmd

    x = np.asarray(x)
    consts = _make_consts(np.asarray(w1r, np.float32), np.asarray(w1i, np.float32),
                          np.asarray(b1, np.float32), np.asarray(w2r, np.float32),
                          np.asarray(w2i, np.float32), np.asarray(b2, np.float32))
    nc = _get_compiled()
    in_maps = []
    for c in range(NCORES):
        m = dict(consts)
        m["xbf"] = np.ascontiguousarray(
            x[:, :, :, c * BS:(c + 1) * BS]).astype(ml_dtypes.bfloat16)
        in_maps.append(m)
    res = run_bass_kernel_spmd(nc, in_maps, core_ids=list(range(NCORES)))
    LAST_RESULT = res
    out = np.concatenate(
        [res.results[c]["out"].astype(np.float32) for c in range(NCORES)], axis=3)
    return out
